# revision 42
# baseline (speedup 1.0000x reference)
"""Distributed Trainium2 kernel for nn_BrocaNetwork (decoder transformer + 180k-vocab head).

Strategy (8 NeuronCores, two SPMD launches):
  1. Body launch (2 cores, one per batch): the 6-layer decoder runs fully
     on-device in bf16 (fp32 accumulation / layernorm stats). Activations are
     feature-major [D, tokens]; attention uses exp-then-causal-mask-multiply
     with per-head ones-column sum rows; cross-attention to the 1-token memory
     collapses exactly (softmax over one key == 1) to a precomputed per-batch
     vector folded into LN1's bias. GEMM loops are k-outer/m-blocked so the
     tensor engine restarts one tile into each layernorm, and layernorm /
     evacuation elementwise work is fused to keep scalar+vector off the
     critical path.
  2. Vocab projection launch (8 cores): w_out is sharded column-wise over the
     vocab (22500 rows/core, padded to 22528); each core computes
     x @ w_out_shard.T for all 1024 tokens in bf16 with fp32 PSUM accumulation.
Host work is limited to input re-layout: embedding row gather, weight
transposes/casts, and output concat.
"""
import sys
sys.path.insert(0, '/opt/trn_rl_repo')
import numpy as np
import ml_dtypes

import concourse.bass as bass
import concourse.mybir as mybir
import concourse.tile as tile
from concourse import bacc
from concourse.bass_utils import run_bass_kernel_spmd

F32 = mybir.dt.float32
BF16 = mybir.dt.bfloat16
Alu = mybir.AluOpType
Act = mybir.ActivationFunctionType
BF = ml_dtypes.bfloat16

B, S, D, H, L, V = 2, 512, 512, 8, 6, 180000
HD = D // H
FF = 4 * D
EPS = 1e-5
DT = D // 128
ST = S // 128
FT = FF // 128
NCORES = 8
VS = 22528              # padded vocab shard (44 * 512); 8 * 22528 >= V
TOK = B * S
NT = VS // 512
MT = TOK // 128


def build_body(n_cores):
    nc = bacc.Bacc("TRN2", target_bir_lowering=False, debug=False, num_devices=n_cores)
    x0T = nc.dram_tensor("x0T", [D, S], BF16, kind="ExternalInput")
    wqT = nc.dram_tensor("wqT", [L, D, D], BF16, kind="ExternalInput")
    wkT = nc.dram_tensor("wkT", [L, D, D], BF16, kind="ExternalInput")
    wvT = nc.dram_tensor("wvT", [L, D, D], BF16, kind="ExternalInput")
    woT = nc.dram_tensor("woT", [L, D, D], BF16, kind="ExternalInput")
    w1T = nc.dram_tensor("w1T", [L, D, FF], BF16, kind="ExternalInput")
    w2T = nc.dram_tensor("w2T", [L, FF, D], BF16, kind="ExternalInput")
    # host-packed per-partition biases: [128, L, 10*DT + FT]
    # order: bq bk abias b2 g1 bl1(+cao) g2 bl2 g3 bl3 (DT each), then b1 (FT)
    NBIAS = 10 * DT + FT
    biasd = nc.dram_tensor("biasd", [128, L, NBIAS], F32, kind="ExternalInput")
    maskd = nc.dram_tensor("maskd", [128, 128], BF16, kind="ExternalInput")
    xfT = nc.dram_tensor("xfT", [D, S], BF16, kind="ExternalOutput")

    with nc.allow_low_precision(reason="bf16 compute kernel"), tile.TileContext(nc) as tc:
        with tc.tile_pool(name="persist", bufs=1) as pp, \
             tc.tile_pool(name="wqkvo", bufs=2) as wpool, \
             tc.tile_pool(name="w1p", bufs=2) as w1pool, \
             tc.tile_pool(name="w2p", bufs=2) as w2pool, \
             tc.tile_pool(name="lntmp", bufs=1) as lnp, \
             tc.tile_pool(name="exps", bufs=12) as xp, \
             tc.tile_pool(name="psB", bufs=5, space="PSUM") as psB, \
             tc.tile_pool(name="psV", bufs=3, space="PSUM") as psV:

            x_res = pp.tile([128, DT, S], F32)
            x_bf = pp.tile([128, DT, S], BF16)
            q_bf = pp.tile([128, DT, S], BF16)
            k_bf = pp.tile([128, DT, S], BF16)
            v_sb = pp.tile([128, ST, H, HD + 1], BF16)
            o_bf = pp.tile([128, DT, S], BF16)
            hh_bf = pp.tile([128, FT, S], BF16)
            ones_bf = pp.tile([128, 128], BF16)
            mask_sb = pp.tile([128, 128], BF16)
            den_tmp = pp.tile([1, H, S], BF16)
            bias_sb = pp.tile([128, L, NBIAS], F32)
            bq_sb = bias_sb[:, :, 0 * DT:1 * DT]
            bk_sb = bias_sb[:, :, 1 * DT:2 * DT]
            ab_sb = bias_sb[:, :, 2 * DT:3 * DT]
            b2_sb = bias_sb[:, :, 3 * DT:4 * DT]
            g1_sb = bias_sb[:, :, 4 * DT:5 * DT]
            bl1_sb = bias_sb[:, :, 5 * DT:6 * DT]
            g2_sb = bias_sb[:, :, 6 * DT:7 * DT]
            bl2_sb = bias_sb[:, :, 7 * DT:8 * DT]
            g3_sb = bias_sb[:, :, 8 * DT:9 * DT]
            bl3_sb = bias_sb[:, :, 9 * DT:10 * DT]
            b1_sb = bias_sb[:, :, 10 * DT:10 * DT + FT]

            eps_sb = pp.tile([128, 1], F32)
            nc.vector.memset(eps_sb[:], float(D * EPS))
            nc.vector.memset(ones_bf[:], 1.0)
            nc.vector.memset(v_sb[:, :, :, HD:HD + 1], 1.0)
            # chunked bf16 input DMA straight into x_bf: first-layer GEMM
            # starts as soon as chunk 0 lands (no cast on the critical path)
            for k in range(DT):
                nc.sync.dma_start(
                    x_bf[:, k, :],
                    x0T.ap().rearrange("(t p) n -> p t n", p=128)[:, k, :])
            nc.sync.dma_start(mask_sb[:], maskd.ap())
            nc.sync.dma_start(bias_sb[:], biasd.ap())

            def blocked_gemm(n_m, w_ap, rhs_tiles, n_k, evac, block=3):
                """out m-tiles via k-chained matmuls, m-blocked (k-outer inside
                a block) so the first matmul depends only on rhs tile 0."""
                m0 = 0
                while m0 < n_m:
                    ms = list(range(m0, min(m0 + block, n_m)))
                    pss = {m: psB.tile([128, S], F32, tag="big", name=f"ps{m}") for m in ms}
                    for k in range(n_k):
                        for m in ms:
                            nc.tensor.matmul(pss[m][:], w_ap(k, m), rhs_tiles(k),
                                             start=(k == 0), stop=(k == n_k - 1))
                    for m in ms:
                        evac(m, pss[m])
                    m0 += block

            def layer_norm(P, Pbf, g_ap, b_ap, out_bf, out_f32=None):
                """P [128,DT,S] f32 (+ Pbf bf16 copy) -> out_bf/out_f32 tiles.
                Stats via ones-matmuls; normalize per k-tile pipelined."""
                Psq = lnp.tile([128, DT, S], BF16, tag="psq")
                for k in range(DT):
                    nc.vector.tensor_mul(Psq[:, k, :], Pbf[:, k, :], Pbf[:, k, :])
                A_ps = psB.tile([128, S], F32, tag="big")
                Q_ps = psB.tile([128, S], F32, tag="big")
                for k in range(DT):
                    nc.tensor.matmul(A_ps[:], ones_bf[:], Pbf[:, k, :], start=(k == 0), stop=(k == DT - 1))
                for k in range(DT):
                    nc.tensor.matmul(Q_ps[:], ones_bf[:], Psq[:, k, :], start=(k == 0), stop=(k == DT - 1))
                t_sb = lnp.tile([128, S], F32, tag="t")
                # critical path to out_bf tile0: Square -> stt -> rsqrt -> h0 -> i0
                nc.scalar.activation(t_sb[:], A_ps[:], Act.Square)
                nc.vector.scalar_tensor_tensor(t_sb[:], t_sb[:], -1.0 / D, Q_ps[:], op0=Alu.mult, op1=Alu.add)
                nc.scalar.activation(t_sb[:], t_sb[:], Act.Abs_reciprocal_sqrt, bias=eps_sb[:])
                for f in range(11):
                    # keep the PE busy across the layernorm bubble (HAM warmth)
                    nc.tensor.matmul(Q_ps[:], ones_bf[:], Pbf[:, 0, :], start=(f == 0), stop=(f == 10))
                nc.vector.scalar_tensor_tensor(P[:, 0, :], A_ps[:], -1.0 / D, P[:, 0, :], op0=Alu.mult, op1=Alu.add)
                for k in range(DT):
                    if k > 0:
                        nc.vector.scalar_tensor_tensor(P[:, k, :], A_ps[:], -1.0 / D, P[:, k, :], op0=Alu.mult, op1=Alu.add)
                    nc.vector.tensor_mul(P[:, k, :], P[:, k, :], t_sb[:])
                    nc.scalar.activation(out_bf[:, k, :], P[:, k, :], Act.Identity, bias=b_ap(k), scale=g_ap(k))
                if out_f32 is not None:
                    for k in range(DT):
                        nc.gpsimd.tensor_scalar(out_f32[:, k, :], P[:, k, :], g_ap(k), b_ap(k), op0=Alu.mult, op1=Alu.add)

            for l in range(L):
                wq_sb = wpool.tile([128, DT, D], BF16, tag="wq")
                wk_sb = wpool.tile([128, DT, D], BF16, tag="wk")
                wv_sb = wpool.tile([128, DT, D], BF16, tag="wv")
                wo_sb = wpool.tile([128, DT, D], BF16, tag="wo")
                w1_sb = w1pool.tile([128, DT, FF], BF16, tag="w1")
                w2_sb = w2pool.tile([128, FT, D], BF16, tag="w2")
                r128 = lambda t: t.ap()[l].rearrange("(t p) n -> p t n", p=128)
                if l == 0:
                    nc.scalar.dma_start(wk_sb[:], r128(wkT))
                    nc.gpsimd.dma_start(wq_sb[:], r128(wqT))
                    nc.gpsimd.dma_start(wv_sb[:], r128(wvT))
                    nc.gpsimd.dma_start(wo_sb[:], r128(woT))
                    nc.gpsimd.dma_start(w1_sb[:, :, 0:FF // 2], r128(w1T)[:, :, 0:FF // 2])
                    nc.scalar.dma_start(w1_sb[:, :, FF // 2:], r128(w1T)[:, :, FF // 2:])
                    nc.sync.dma_start(w2_sb[:], r128(w2T))
                else:
                    nc.sync.dma_start(wk_sb[:], r128(wkT))
                    nc.sync.dma_start(wq_sb[:], r128(wqT))
                    nc.gpsimd.dma_start(wv_sb[:], r128(wvT))
                    nc.gpsimd.dma_start(wo_sb[:], r128(woT))
                    nc.gpsimd.dma_start(w1_sb[:], r128(w1T))
                    nc.sync.dma_start(w2_sb[:], r128(w2T))

                # K and Q projections (8 m-tiles: k0..3, q0..3)
                def kq_w(k, m):
                    if m < DT:
                        return wk_sb[:, k, m * 128:(m + 1) * 128]
                    return wq_sb[:, k, (m - DT) * 128:(m - DT + 1) * 128]

                def kq_evac(m, ps):
                    if m < DT:
                        nc.scalar.activation(k_bf[:, m, :], ps[:], Act.Identity, bias=bk_sb[:, l, m:m + 1])
                    else:
                        nc.scalar.activation(q_bf[:, m - DT, :], ps[:], Act.Identity, bias=bq_sb[:, l, m - DT:m - DT + 1])

                blocked_gemm(2 * DT, kq_w, lambda k: x_bf[:, k, :], DT, kq_evac)

                # V projection (token-major out; x tile stationary)
                def v_w(k, t):
                    return x_bf[:, k, t * 128:(t + 1) * 128]

                def v_evac(t, ps):
                    nc.vector.tensor_copy(v_sb[:, t, :, 0:HD], ps[:].rearrange("p (h d) -> p h d", h=H))

                blocked_gemm(ST, v_w, lambda k: wv_sb[:, k, :], DT, v_evac)

                # attention
                av_tiles = [None] * H

                def head_norm(h):
                    """normalize head h -> its 64 rows of o_bf (deferred by 2 heads)"""
                    dt_i = h // 2
                    pb = 64 * (h % 2)
                    bc_ps = psB.tile([HD, S], F32, tag="big", name="bc_ps")
                    nc.tensor.matmul(bc_ps[:], ones_bf[0:1, 0:HD], den_tmp[0:1, h, :],
                                     start=True, stop=True)
                    rc = lnp.tile([HD, S], F32, tag="rc", name="rc", bufs=2)
                    nc.vector.reciprocal_approx_fast(out=rc[:], in_=bc_ps[:])
                    nc.vector.tensor_mul(o_bf[pb:pb + HD, dt_i, :], av_tiles[h][0:HD, :], rc[:])

                sc_exs = {}

                def sc_block(h):
                    dt_i = h // 2
                    pb = 64 * (h % 2)
                    exs = []
                    for i in range(ST):
                        w = S - 128 * i
                        sc = psB.tile([128, S], F32, tag="big", name="sc")
                        nc.tensor.matmul(sc[:, 0:w],
                                         k_bf[pb:pb + HD, dt_i, i * 128:(i + 1) * 128],
                                         q_bf[pb:pb + HD, dt_i, i * 128:S],
                                         start=True, stop=True)
                        ex = xp.tile([128, S], BF16, tag="ex")
                        exs.append(ex)
                        nc.scalar.activation(ex[:, 0:w], sc[:, 0:w], Act.Exp, scale=1.0 / np.sqrt(HD))
                        nc.vector.tensor_mul(ex[:, 0:128], ex[:, 0:128], mask_sb[:])
                    sc_exs[h] = exs

                def av_block(h):
                    av_ps = psV.tile([HD + 1, S], F32, tag="av", name="av_ps")
                    av_tiles[h] = av_ps
                    exs = sc_exs[h]
                    for i in range(ST):
                        w = S - 128 * i
                        nc.tensor.matmul(av_ps[:, i * 128:S], v_sb[:, i, h, :], exs[i][:, 0:w],
                                         start=(i == 0), stop=(i == ST - 1), skip_group_check=True)
                    fill = psB.tile([128, S], F32, tag="big", name="fill")
                    for f in range(2):
                        # PE filler across the scalar-bound exp phase (HAM warmth)
                        nc.tensor.matmul(fill[:], ones_bf[:], x_bf[:, 0, :], start=(f == 0), stop=(f == 1))
                    nc.scalar.copy(den_tmp[0:1, h, :], av_ps[HD:HD + 1, :])

                # 2-head software pipeline: scores(h) run while exp/mask of
                # h-1 and av of h-2 catch up; PE never waits on the scalar exp
                for h in range(H):
                    sc_block(h)
                    if h >= 2:
                        av_block(h - 2)
                    if h >= 3:
                        head_norm(h - 3)
                for h in (H - 2, H - 1):
                    av_block(h)
                for h in (H - 3, H - 2, H - 1):
                    head_norm(h)

                # O projection + residual -> P1
                P1 = lnp.tile([128, DT, S], F32, tag="P1")
                P1bf = lnp.tile([128, DT, S], BF16, tag="P1bf")

                res_in = x_bf if l == 0 else x_res

                def o_evac(m, ps):
                    nc.vector.scalar_tensor_tensor(P1[:, m, :], ps[:], ab_sb[:, l, m:m + 1],
                                                   res_in[:, m, :], op0=Alu.add, op1=Alu.add)
                    nc.scalar.activation(P1bf[:, m, :], P1[:, m, :], Act.Identity)

                blocked_gemm(DT, lambda k, m: wo_sb[:, k, m * 128:(m + 1) * 128],
                             lambda k: o_bf[:, k, :], DT, o_evac)

                # LN1 with cao folded into bias -> P2 (f32) + P2bf
                P2 = lnp.tile([128, DT, S], F32, tag="P2")
                P2bf = lnp.tile([128, DT, S], BF16, tag="P2bf")
                layer_norm(P1, P1bf, lambda k: g1_sb[:, l, k:k + 1], lambda k: bl1_sb[:, l, k:k + 1],
                           P2bf, P2)

                # LN2 -> x_bf / x_res
                layer_norm(P2, P2bf, lambda k: g2_sb[:, l, k:k + 1], lambda k: bl2_sb[:, l, k:k + 1],
                           x_bf, x_res)

                # FFN w1 (+relu), evacuation split across scalar/vector
                def w1_evac(m, ps):
                    if m % 4 != 3:
                        nc.scalar.activation(hh_bf[:, m, :], ps[:], Act.Relu, bias=b1_sb[:, l, m:m + 1])
                    else:
                        nc.vector.tensor_scalar(hh_bf[:, m, :], ps[:], b1_sb[:, l, m:m + 1], 0.0,
                                                op0=Alu.add, op1=Alu.max)

                blocked_gemm(FT, lambda k, m: w1_sb[:, k, m * 128:(m + 1) * 128],
                             lambda k: x_bf[:, k, :], DT, w1_evac)

                # FFN w2 + residual -> P3
                P3 = lnp.tile([128, DT, S], F32, tag="P1")
                P3bf = lnp.tile([128, DT, S], BF16, tag="P1bf")

                def w2_evac(m, ps):
                    nc.vector.scalar_tensor_tensor(P3[:, m, :], ps[:], b2_sb[:, l, m:m + 1],
                                                   x_res[:, m, :], op0=Alu.add, op1=Alu.add)
                    nc.scalar.activation(P3bf[:, m, :], P3[:, m, :], Act.Identity)

                blocked_gemm(DT, lambda k, m: w2_sb[:, k, m * 128:(m + 1) * 128],
                             lambda k: hh_bf[:, k, :], FT, w2_evac)

                # LN3 -> x_bf / x_res (last layer: bf16 output only)
                layer_norm(P3, P3bf, lambda k: g3_sb[:, l, k:k + 1], lambda k: bl3_sb[:, l, k:k + 1],
                           x_bf, x_res if l < L - 1 else None)

            for k in range(DT):
                nc.sync.dma_start(
                    xfT.ap().rearrange("(t p) n -> p t n", p=128)[:, k, :], x_bf[:, k, :])
    nc.compile()
    return nc


def build_proj(n_cores):
    nc = bacc.Bacc("TRN2", target_bir_lowering=False, debug=False, num_devices=n_cores)
    xT = nc.dram_tensor("xT", [D, TOK], BF16, kind="ExternalInput")
    wT = nc.dram_tensor("wT", [D, VS], BF16, kind="ExternalInput")
    out = nc.dram_tensor("out", [TOK, VS], BF16, kind="ExternalOutput")
    with tile.TileContext(nc) as tc:
        with tc.tile_pool(name="xp", bufs=1) as xpool, \
             tc.tile_pool(name="wp", bufs=8) as wpool, \
             tc.tile_pool(name="op", bufs=6) as opool, \
             tc.tile_pool(name="ps", bufs=8, space="PSUM") as psp:
            x_sb = xpool.tile([128, DT, TOK], BF16)
            for k in range(DT):
                nc.sync.dma_start(
                    x_sb[:, k, :], xT.ap().rearrange("(t p) n -> p t n", p=128)[:, k, :])
            warm = psp.tile([128, 512], F32, tag="ps")
            for f in range(20):
                nc.tensor.matmul(warm[:], x_sb[:, 0, 0:128], x_sb[:, 0, 0:512],
                                 start=(f == 0), stop=(f == 19))
            for nt in range(NT):
                w_sb = wpool.tile([128, DT, 512], BF16)
                nc.gpsimd.dma_start(w_sb[:], wT.ap()[:, nt * 512:(nt + 1) * 512].rearrange("(t p) n -> p t n", p=128))
                for mt in range(MT):
                    ps = psp.tile([128, 512], F32, tag="ps")
                    for kt in range(DT):
                        nc.tensor.matmul(ps[:], x_sb[:, kt, mt * 128:(mt + 1) * 128], w_sb[:, kt, :],
                                         start=(kt == 0), stop=(kt == DT - 1))
                    ob = opool.tile([128, 512], BF16)
                    if mt % 2 == 0:
                        nc.scalar.copy(ob[:], ps[:])
                    else:
                        nc.vector.tensor_copy(ob[:], ps[:])
                    nc.sync.dma_start(out.ap()[mt * 128:(mt + 1) * 128, nt * 512:(nt + 1) * 512], ob[:])
    nc.compile()
    return nc


_CACHE = {}


def _get_kernels():
    if "body" not in _CACHE:
        _CACHE["body"] = build_body(B)
        _CACHE["proj"] = build_proj(NCORES)
    return _CACHE["body"], _CACHE["proj"]


def _body_in_map(x0, sa_wq, sa_wk, sa_wv, sa_bq, sa_bk, sa_bv, sa_wo, sa_bo,
                 ca_wv, ca_bv, ca_wo, ca_bo, meaning_b,
                 ln1_g, ln1_b, ln2_g, ln2_b, ln3_g, ln3_b,
                 ff_w1, ff_b1, ff_w2, ff_b2, mask):
    tp = lambda w: np.ascontiguousarray(w.transpose(0, 2, 1)).astype(BF)
    cao = np.stack([
        (meaning_b @ ca_wv[l].T + ca_bv[l]) @ ca_wo[l].T + ca_bo[l] for l in range(L)])
    abias = np.stack([sa_bo[l] + sa_wo[l] @ sa_bv[l] for l in range(L)])
    sq = np.float32(np.sqrt(D))
    f32 = lambda a: np.ascontiguousarray(a, dtype=np.float32)
    def part(a):
        # [L, D] -> [128, L, DT]: dev[p, l, t] = a[l, t*128+p]
        return np.ascontiguousarray(np.asarray(a, np.float32).reshape(L, -1, 128).transpose(2, 0, 1))
    packed = np.concatenate([
        part(sa_bq), part(sa_bk), part(abias), part(ff_b2),
        part(ln1_g * sq), part(ln1_b + cao), part(ln2_g * sq), part(ln2_b),
        part(ln3_g * sq), part(ln3_b), part(ff_b1)], axis=2)
    return {
        "x0T": np.ascontiguousarray(x0.T).astype(BF),
        "wqT": tp(sa_wq), "wkT": tp(sa_wk), "wvT": tp(sa_wv), "woT": tp(sa_wo),
        "w1T": tp(ff_w1), "w2T": tp(ff_w2),
        "biasd": np.ascontiguousarray(packed),
        "maskd": mask,
    }


def kernel(meaning, target_ids, emb_table, pos_table,
           sa_wq, sa_wk, sa_wv, sa_bq, sa_bk, sa_bv, sa_wo, sa_bo,
           ca_wq, ca_wk, ca_wv, ca_bq, ca_bk, ca_bv, ca_wo, ca_bo,
           ln1_g, ln1_b, ln2_g, ln2_b, ln3_g, ln3_b,
           ff_w1, ff_b1, ff_w2, ff_b2, w_out, b_out):
    meaning = np.asarray(meaning, dtype=np.float32)
    target_ids = np.asarray(target_ids)
    emb_table = np.asarray(emb_table, dtype=np.float32)
    pos_table = np.asarray(pos_table, dtype=np.float32)

    body_nc, proj_nc = _get_kernels()

    mask = (np.arange(128)[:, None] <= np.arange(128)[None, :]).astype(BF)
    in_maps = []
    for b in range(B):
        x0 = emb_table[target_ids[b]] + pos_table[:S]
        in_maps.append(_body_in_map(
            x0, np.asarray(sa_wq, np.float32), np.asarray(sa_wk, np.float32),
            np.asarray(sa_wv, np.float32), np.asarray(sa_bq, np.float32),
            np.asarray(sa_bk, np.float32), np.asarray(sa_bv, np.float32),
            np.asarray(sa_wo, np.float32), np.asarray(sa_bo, np.float32),
            np.asarray(ca_wv, np.float32), np.asarray(ca_bv, np.float32),
            np.asarray(ca_wo, np.float32), np.asarray(ca_bo, np.float32),
            meaning[b],
            np.asarray(ln1_g, np.float32), np.asarray(ln1_b, np.float32),
            np.asarray(ln2_g, np.float32), np.asarray(ln2_b, np.float32),
            np.asarray(ln3_g, np.float32), np.asarray(ln3_b, np.float32),
            np.asarray(ff_w1, np.float32), np.asarray(ff_b1, np.float32),
            np.asarray(ff_w2, np.float32), np.asarray(ff_b2, np.float32), mask))
    body_res = run_bass_kernel_spmd(body_nc, in_maps, core_ids=list(range(B)))
    xT_all = np.concatenate([body_res.results[b]["xfT"] for b in range(B)], axis=1)  # [D, TOK] bf16

    w_out = np.asarray(w_out, dtype=np.float32)
    wT_pad = np.zeros((D, NCORES * VS), dtype=BF)
    wT_pad[:, :V] = w_out.T.astype(BF)
    proj_maps = [{"xT": xT_all, "wT": np.ascontiguousarray(wT_pad[:, c * VS:(c + 1) * VS])}
                 for c in range(NCORES)]
    proj_res = run_bass_kernel_spmd(proj_nc, proj_maps, core_ids=list(range(NCORES)))

    logits = np.empty((TOK, V), dtype=np.float32)
    for c in range(NCORES):
        lo = c * VS
        hi = min(lo + VS, V)
        if hi > lo:
            logits[:, lo:hi] = proj_res.results[c]["out"][:, :hi - lo].astype(np.float32)
    b_out = np.asarray(b_out, dtype=np.float32)
    if np.any(b_out):
        logits += b_out[None, :]
    return logits.reshape(B, S, V)
